# revision 1
# baseline (speedup 1.0000x reference)
"""AliasMHA kernel: data-parallel over batch B across 8 NeuronCores.

Self-contained. Accepts FULL unsharded inputs, shards batch across the 8
axon-tunneled trn2 NeuronCores (16 batch elements per core), runs the two
post-norm transformer attention blocks per shard, and gathers the full
output. Falls back to CPU execution if the neuron devices are unavailable.
"""

import numpy as np
import jax
import jax.numpy as jnp

# Problem shapes (hardcoded per spec): B,N,M,K,D,F,H
B, N, M, K, D, F, H = 128, 128, 24, 32, 512, 1024, 8
NEG = -1e9
N_CORES = 8


def _layer_norm(x, g, b, eps=1e-5):
    mu = x.mean(-1, keepdims=True)
    var = ((x - mu) ** 2).mean(-1, keepdims=True)
    return (x - mu) * jax.lax.rsqrt(var + eps) * g + b


def _attn_block(q, x, key_mask, attn_mask, p):
    b, lq, d = q.shape
    lk = x.shape[1]
    dh = d // H
    qh = (q @ p["Wq"] + p["bq"]).reshape(b, lq, H, dh).transpose(0, 2, 1, 3)
    kh = (x @ p["Wk"] + p["bk"]).reshape(b, lk, H, dh).transpose(0, 2, 1, 3)
    vh = (x @ p["Wv"] + p["bv"]).reshape(b, lk, H, dh).transpose(0, 2, 1, 3)
    scale = np.float32(1.0 / np.sqrt(dh))
    scores = jnp.einsum("bhqd,bhkd->bhqk", qh, kh) * scale
    if attn_mask is not None:
        scores = scores + attn_mask
    scores = jnp.where(key_mask[:, None, None, :], jnp.float32(NEG), scores)
    w = jax.nn.softmax(scores, axis=-1)
    ctx = jnp.einsum("bhqk,bhkd->bhqd", w, vh).transpose(0, 2, 1, 3).reshape(b, lq, d)
    ctx = ctx @ p["Wo"] + p["bo"]
    res = _layer_norm(q + ctx, p["ln1_g"], p["ln1_b"])
    ff = jax.nn.relu(res @ p["W1"] + p["b1"]) @ p["W2"] + p["b2"]
    out = _layer_norm(res + ff, p["ln2_g"], p["ln2_b"])
    return out, w.mean(axis=1)


def _forward(sent_tensor, sent_mask, entity_embedding, entity_mask,
             alias_start, slice_emb_alias, slice_emb_ent, sa_params, ea_params):
    b, m, k, d = entity_embedding.shape
    aw = jnp.take_along_axis(sent_tensor, alias_start[:, :, None], axis=1)
    aw = aw + slice_emb_alias
    aw, alias_word_weights = _attn_block(aw, sent_tensor, sent_mask, None, sa_params)
    aw = aw + slice_emb_ent
    ent = entity_embedding.reshape(b, m * k, d)
    ent_key_mask = entity_mask.reshape(b, m * k)
    block = jnp.repeat(jnp.eye(m, dtype=jnp.float32), k, axis=1)
    attn_mask = (1.0 - block) * jnp.float32(NEG)
    ctx, _ = _attn_block(aw, ent, ent_key_mask, attn_mask, ea_params)
    return ctx, alias_word_weights


def kernel(sent_tensor, sent_mask, entity_embedding, entity_mask,
           alias_start, slice_emb_alias, slice_emb_ent, sa_params, ea_params):
    alias_start = np.asarray(alias_start).astype(np.int32)
    sent_tensor = np.asarray(sent_tensor, np.float32)
    entity_embedding = np.asarray(entity_embedding, np.float32)
    sent_mask = np.asarray(sent_mask, bool)
    entity_mask = np.asarray(entity_mask, bool)
    slice_emb_alias = np.asarray(slice_emb_alias, np.float32)
    slice_emb_ent = np.asarray(slice_emb_ent, np.float32)
    sa_params = {k_: np.asarray(v, np.float32) for k_, v in sa_params.items()}
    ea_params = {k_: np.asarray(v, np.float32) for k_, v in ea_params.items()}

    b_full = sent_tensor.shape[0]

    try:
        devs = jax.devices()
        nd = min(N_CORES, len(devs), b_full)
        while b_full % nd != 0:
            nd -= 1
        per = b_full // nd
        fn = jax.pmap(
            _forward,
            in_axes=(0, 0, 0, 0, 0, None, None, None, None),
            devices=devs[:nd],
        )
        rs = lambda a: a.reshape((nd, per) + a.shape[1:])
        ctx, w = fn(rs(sent_tensor), rs(sent_mask), rs(entity_embedding),
                    rs(entity_mask), rs(alias_start),
                    slice_emb_alias, slice_emb_ent, sa_params, ea_params)
        ctx = np.asarray(ctx, np.float32).reshape((b_full,) + ctx.shape[2:])
        w = np.asarray(w, np.float32).reshape((b_full,) + w.shape[2:])
        return ctx, w
    except Exception:
        cpu = jax.devices("cpu")[0]
        with jax.default_device(cpu):
            ctx, w = jax.jit(_forward)(
                sent_tensor, sent_mask, entity_embedding, entity_mask,
                alias_start, slice_emb_alias, slice_emb_ent, sa_params, ea_params)
            return np.asarray(ctx, np.float32), np.asarray(w, np.float32)
